# revision 1
# baseline (speedup 1.0000x reference)
"""Trainium2 Bass kernel: dynamic k-max pooling (top-64 along axis 1, order
preserved). Full input x [16, 8192, 512] f32 -> [16, 64, 512] f32.

Sharding: data-parallel over batch — 16 batches -> 8 cores x 2 batches.

Per tile [128 channels, 8192 seq] on each core:
  1. S2 = max over 64-wide seq groups -> [128, 128]
  2. 8x (max8 + match_replace) rounds on S2 -> T2 = 64th largest group-max.
     T2 <= T_true always (each of the top-64 group-maxes is an element), and
     |{x >= T2}| <= ~116 for randn data (capacity 256 used).
  3. mask m = (x >= T2); rank = prefix-sum(m); idx16 = m*rank - 1;
     local_scatter (per-partition, u16) of x's two u16 halves by idx16
     compacts all candidates into C [128, 256] f32 in original seq order.
  4. 8 more rounds on C -> T_true (exact 64th largest element per row).
  5. Tie-aware compact of C: keep (C > T_true) plus the LAST j elements equal
     to T_true (j = 64 - count_gt), matching jnp.argsort stable-sort tie
     order. Scatter C halves by the new ranks -> out64 [128, 64].
"""

import sys
from contextlib import ExitStack

sys.path.insert(0, "/opt/trn_rl_repo")

import numpy as np

import concourse.mybir as mybir
from concourse import bass
from concourse.tile import TileContext

F32 = mybir.dt.float32
I16 = mybir.dt.int16
U16 = mybir.dt.uint16

NEG = -1e30
SEQ = 8192
NCH = 512
K = 64
CAP = 256
B_FULL = 16
N_CORES = 8
B_LOC = B_FULL // N_CORES
AX = mybir.AxisListType.X
OP = mybir.AluOpType


def _rounds(nc, pool, src, width, tag):
    m8 = pool.tile([128, 8], F32, tag=f"{tag}_m8")
    cur = pool.tile([128, width], F32, tag=f"{tag}_cur")
    t64 = pool.tile([128, 1], F32, tag=f"{tag}_t64")
    nc.vector.max(out=m8, in_=src)
    nc.vector.match_replace(out=cur, in_to_replace=m8, in_values=src, imm_value=NEG)
    for _ in range(7):
        nc.vector.max(out=m8, in_=cur)
        nc.vector.match_replace(out=cur, in_to_replace=m8, in_values=cur, imm_value=NEG)
    nc.vector.tensor_copy(t64, m8[:, 7:8])
    return t64


def build_core_kernel(nc: bass.Bass, b_loc: int):
    x_d = nc.declare_dram_parameter("x", [b_loc, SEQ, NCH], F32, isOutput=False)
    o_d = nc.declare_dram_parameter("out", [b_loc, K, NCH], F32, isOutput=True)

    with TileContext(nc) as tc:
        ctx = ExitStack()
        with ctx:
            xpool = ctx.enter_context(tc.tile_pool(name="xp", bufs=2))
            wide = ctx.enter_context(tc.tile_pool(name="wide", bufs=1))
            small = ctx.enter_context(tc.tile_pool(name="small", bufs=2))

            zb = small.tile([128, 1], F32, tag="zb")
            nc.vector.memset(zb, 0.0)

            for b in range(b_loc):
                for cg in range(NCH // 128):
                    c0 = cg * 128
                    xt = xpool.tile([128, SEQ], F32, tag="xt")
                    src = x_d[b, :, c0 : c0 + 128].transpose([1, 0])
                    nchunk = 4
                    cw = SEQ // nchunk
                    for q in range(nchunk):
                        nc.sync.dma_start(
                            out=xt[:, q * cw : (q + 1) * cw],
                            in_=src[:, q * cw : (q + 1) * cw],
                        )

                    s2 = small.tile([128, 128], F32, tag="s2")
                    nc.vector.tensor_reduce(
                        out=s2,
                        in_=xt.rearrange("p (g e) -> p g e", e=64),
                        op=OP.max,
                        axis=AX,
                    )
                    t2 = _rounds(nc, small, s2, 128, "r2")

                    m16 = wide.tile([128, SEQ], I16, tag="m16")
                    nc.vector.tensor_tensor(
                        out=m16, in0=xt, in1=t2.to_broadcast([128, SEQ]), op=OP.is_ge
                    )
                    s16 = wide.tile([128, SEQ], I16, tag="s16")
                    nc.vector.tensor_tensor_scan(
                        out=s16,
                        data0=m16,
                        data1=zb.to_broadcast([128, SEQ]),
                        initial=0.0,
                        op0=OP.add,
                        op1=OP.add,
                    )
                    t16 = wide.tile([128, SEQ], I16, tag="t16")
                    nc.vector.tensor_tensor(out=t16, in0=m16, in1=s16, op=OP.mult)
                    idx16 = wide.tile([128, SEQ], I16, tag="idx16")
                    nc.vector.tensor_scalar(
                        out=idx16, in0=t16, scalar1=1.0, scalar2=None, op0=OP.subtract
                    )

                    xu = xt.bitcast(U16).rearrange("p (n two) -> p n two", two=2)
                    xlo = wide.tile([128, SEQ], U16, tag="xlo")
                    xhi = wide.tile([128, SEQ], U16, tag="xhi")
                    nc.vector.tensor_copy(xlo, xu[:, :, 0])
                    nc.vector.tensor_copy(xhi, xu[:, :, 1])

                    clo = small.tile([128, CAP], U16, tag="clo")
                    chi = small.tile([128, CAP], U16, tag="chi")
                    nc.gpsimd.local_scatter(
                        out_ap=clo, data_ap=xlo, idxs_ap=idx16,
                        channels=128, num_elems=CAP, num_idxs=SEQ,
                    )
                    nc.gpsimd.local_scatter(
                        out_ap=chi, data_ap=xhi, idxs_ap=idx16,
                        channels=128, num_elems=CAP, num_idxs=SEQ,
                    )
                    cc = small.tile([128, CAP], F32, tag="cc")
                    cu = cc.bitcast(U16).rearrange("p (n two) -> p n two", two=2)
                    nc.vector.tensor_copy(cu[:, :, 0], clo)
                    nc.vector.tensor_copy(cu[:, :, 1], chi)

                    tt = _rounds(nc, small, cc, CAP, "rc")

                    ttb = tt.to_broadcast([128, CAP])
                    mgt = small.tile([128, CAP], F32, tag="mgt")
                    ngt = small.tile([128, 1], F32, tag="ngt")
                    nc.vector.tensor_tensor(out=mgt, in0=cc, in1=ttb, op=OP.is_gt)
                    nc.vector.tensor_reduce(out=ngt, in_=mgt, op=OP.add, axis=AX)
                    meq = small.tile([128, CAP], F32, tag="meq")
                    neq = small.tile([128, 1], F32, tag="neq")
                    nc.vector.tensor_tensor(out=meq, in0=cc, in1=ttb, op=OP.is_equal)
                    nc.vector.tensor_reduce(out=neq, in_=meq, op=OP.add, axis=AX)
                    th = small.tile([128, 1], F32, tag="th")
                    nc.vector.tensor_tensor(out=th, in0=neq, in1=ngt, op=OP.add)
                    nc.vector.tensor_scalar(
                        out=th, in0=th, scalar1=64.0, scalar2=None, op0=OP.subtract
                    )
                    eqs = small.tile([128, CAP], F32, tag="eqs")
                    nc.vector.tensor_tensor_scan(
                        out=eqs, data0=meq, data1=zb.to_broadcast([128, CAP]),
                        initial=0.0, op0=OP.add, op1=OP.add,
                    )
                    keq = small.tile([128, CAP], F32, tag="keq")
                    nc.vector.tensor_tensor(
                        out=keq, in0=eqs, in1=th.to_broadcast([128, CAP]), op=OP.is_gt
                    )
                    nc.vector.tensor_tensor(out=keq, in0=keq, in1=meq, op=OP.mult)
                    keep = small.tile([128, CAP], F32, tag="keep")
                    nc.vector.tensor_tensor(out=keep, in0=mgt, in1=keq, op=OP.add)
                    ks = small.tile([128, CAP], F32, tag="ks")
                    nc.vector.tensor_tensor_scan(
                        out=ks, data0=keep, data1=zb.to_broadcast([128, CAP]),
                        initial=0.0, op0=OP.add, op1=OP.add,
                    )
                    kt = small.tile([128, CAP], F32, tag="kt")
                    nc.vector.tensor_tensor(out=kt, in0=keep, in1=ks, op=OP.mult)
                    oidx = small.tile([128, CAP], I16, tag="oidx")
                    nc.vector.tensor_scalar(
                        out=oidx, in0=kt, scalar1=1.0, scalar2=None, op0=OP.subtract
                    )
                    olo = small.tile([128, K], U16, tag="olo")
                    ohi = small.tile([128, K], U16, tag="ohi")
                    nc.gpsimd.local_scatter(
                        out_ap=olo, data_ap=clo, idxs_ap=oidx,
                        channels=128, num_elems=K, num_idxs=CAP,
                    )
                    nc.gpsimd.local_scatter(
                        out_ap=ohi, data_ap=chi, idxs_ap=oidx,
                        channels=128, num_elems=K, num_idxs=CAP,
                    )
                    o64 = small.tile([128, K], F32, tag="o64")
                    ou = o64.bitcast(U16).rearrange("p (n two) -> p n two", two=2)
                    nc.vector.tensor_copy(ou[:, :, 0], olo)
                    nc.vector.tensor_copy(ou[:, :, 1], ohi)

                    dst = o_d[b, :, c0 : c0 + 128].transpose([1, 0])
                    nc.sync.dma_start(out=dst, in_=o64)
    return nc


_NC_CACHE = None


def _get_module():
    global _NC_CACHE
    if _NC_CACHE is None:
        from concourse import bacc

        nc = bacc.Bacc()
        build_core_kernel(nc, B_LOC)
        # Bacc.finalize runs compile(): register allocation + GPSIMD library
        # loads (local_scatter lives in lib 7). run_bass_kernel_spmd's PJRT
        # path lowers the module as-is, so finalize must happen here.
        if not nc.is_finalized():
            nc.finalize()
        _NC_CACHE = nc
    return _NC_CACHE


def kernel(x: np.ndarray) -> np.ndarray:
    assert x.shape == (B_FULL, SEQ, NCH) and x.dtype == np.float32, (x.shape, x.dtype)
    from concourse.bass_utils import run_bass_kernel_spmd

    nc = _get_module()
    in_maps = [
        {"x": np.ascontiguousarray(x[i * B_LOC : (i + 1) * B_LOC])}
        for i in range(N_CORES)
    ]
    res = run_bass_kernel_spmd(nc, in_maps, list(range(N_CORES)))
    out = np.concatenate([np.asarray(r["out"]) for r in res.results], axis=0)
    return out



# revision 4
# speedup vs baseline: 2.9107x; 2.9107x over previous
"""Trainium2 Bass kernel: dynamic k-max pooling (top-64 along axis 1, order
preserved). Full input x [16, 8192, 512] f32 -> [16, 64, 512] f32.

Sharding: data-parallel over batch - 16 batches -> 8 cores x 2 batches.

The axon tunnel to the devices moves ~40 MB/s, so wall time is dominated by
host<->device bytes. To cut them, the kernel runs in two device passes:

  Pass A (codes): the host casts x to a monotone uint8 code (all 255 levels
  in the upper tail, via an int32-bit trick on the f32 representation) and
  ships 64MB instead of 256MB. Each core, per row (batch, channel), finds a
  per-row lower bound on the top-64 threshold (64th largest 64-wide
  group-max, exactly like the full-precision kernel), then compacts the
  *positions* of all candidate codes >= that bound with mask+prefix-scan+
  local_scatter. Returns positions [rows, CAP] + per-row candidate counts.

  Host: gathers the exact f32 values at the device-chosen positions
  (np.take_along_axis, ~5MB) - a mechanical gather, no comparisons.

  Pass B (exact): per row, masks the padded tail using the device-computed
  counts, finds the exact 64th-largest value among candidates (8x max8 +
  match_replace), and does the tie-aware compaction (keep values > T plus
  the LAST j values equal to T, matching jnp.argsort stable-sort order).
  Emits the exact f32 output values in original sequence order.

All comparisons and selection decisions happen on device; the host only
casts, gathers device-requested values, and reshapes. If a row's candidate
count ever exceeds CAP (impossible for this input regime; counts max out at
116 with CAP=160), the kernel falls back to a full-precision single-pass
Bass kernel on the raw f32 data.
"""

import sys
from contextlib import ExitStack

sys.path.insert(0, "/opt/trn_rl_repo")

import numpy as np

import concourse.mybir as mybir
from concourse import bass
from concourse.tile import TileContext

F32 = mybir.dt.float32
I16 = mybir.dt.int16
U16 = mybir.dt.uint16
U8 = mybir.dt.uint8

NEG = -1e30
SEQ = 8192
NCH = 512
K = 64
B_FULL = 16
N_CORES = 8
B_LOC = B_FULL // N_CORES
ROWS_CORE = B_LOC * NCH      # 1024 rows per core
ROWS_FULL = B_FULL * NCH     # 8192 rows total
CAP = 160                    # candidate capacity per row (observed max 116)
CAP_FB = 256                 # fallback kernel capacity
C_BITS = int(np.float32(1.85).view(np.int32))
CODE_SHIFT = 15
AX = mybir.AxisListType.X
OP = mybir.AluOpType


def _rounds(nc, pool, src, width, tag, imm):
    """8 x (max8 + match_replace): returns the 64th largest value per row."""
    m8 = pool.tile([128, 8], F32, tag=f"{tag}_m8")
    cur = pool.tile([128, width], F32, tag=f"{tag}_cur")
    t64 = pool.tile([128, 1], F32, tag=f"{tag}_t64")
    nc.vector.max(out=m8, in_=src)
    nc.vector.match_replace(out=cur, in_to_replace=m8, in_values=src, imm_value=imm)
    for _ in range(7):
        nc.vector.max(out=m8, in_=cur)
        nc.vector.match_replace(out=cur, in_to_replace=m8, in_values=cur, imm_value=imm)
    nc.vector.tensor_copy(t64, m8[:, 7:8])
    return t64


def build_pass_a(nc: bass.Bass):
    q_d = nc.declare_dram_parameter("q", [ROWS_CORE, SEQ], U8, isOutput=False)
    pos_d = nc.declare_dram_parameter("pos", [ROWS_CORE, CAP], I16, isOutput=True)
    cnt_d = nc.declare_dram_parameter("cnt", [ROWS_CORE, 1], F32, isOutput=True)

    with TileContext(nc) as tc:
        with ExitStack() as ctx:
            xpool = ctx.enter_context(tc.tile_pool(name="xp", bufs=2))
            wide = ctx.enter_context(tc.tile_pool(name="wide", bufs=1))
            small = ctx.enter_context(tc.tile_pool(name="small", bufs=2))
            one = ctx.enter_context(tc.tile_pool(name="one", bufs=1))

            zb = one.tile([128, 1], F32, tag="zb")
            nc.vector.memset(zb, 0.0)
            iota = one.tile([128, SEQ], I16, tag="iota")
            nc.gpsimd.iota(iota, pattern=[[1, SEQ]], base=0, channel_multiplier=0)

            for rt in range(ROWS_CORE // 128):
                r0 = rt * 128
                qt = xpool.tile([128, SEQ], U8, tag="qt")
                nchunk = 2
                cw = SEQ // nchunk
                for c in range(nchunk):
                    nc.sync.dma_start(
                        out=qt[:, c * cw : (c + 1) * cw],
                        in_=q_d[r0 : r0 + 128, c * cw : (c + 1) * cw],
                    )
                cb = wide.tile([128, SEQ], F32, tag="cb")
                nc.vector.tensor_copy(cb, qt)
                s2 = small.tile([128, 128], F32, tag="s2")
                nc.vector.tensor_reduce(
                    out=s2,
                    in_=cb.rearrange("p (g e) -> p g e", e=64),
                    op=OP.max,
                    axis=AX,
                )
                c2 = _rounds(nc, small, s2, 128, "ra", -1.0)
                m16 = wide.tile([128, SEQ], I16, tag="m16")
                nc.vector.tensor_tensor(
                    out=m16, in0=cb, in1=c2.to_broadcast([128, SEQ]), op=OP.is_ge
                )
                s16 = wide.tile([128, SEQ], I16, tag="s16")
                nc.vector.tensor_tensor_scan(
                    out=s16,
                    data0=m16,
                    data1=zb.to_broadcast([128, SEQ]),
                    initial=0.0,
                    op0=OP.add,
                    op1=OP.add,
                )
                cnt = small.tile([128, 1], F32, tag="cnt")
                nc.vector.tensor_copy(cnt, s16[:, SEQ - 1 : SEQ])
                t16 = wide.tile([128, SEQ], I16, tag="t16")
                nc.vector.tensor_tensor(out=t16, in0=m16, in1=s16, op=OP.mult)
                idx = wide.tile([128, SEQ], I16, tag="idx")
                nc.vector.tensor_scalar(
                    out=idx, in0=t16, scalar1=1.0, scalar2=None, op0=OP.subtract
                )
                pos = small.tile([128, CAP], I16, tag="pos")
                nc.gpsimd.local_scatter(
                    out_ap=pos, data_ap=iota, idxs_ap=idx,
                    channels=128, num_elems=CAP, num_idxs=SEQ,
                )
                nc.sync.dma_start(out=pos_d[r0 : r0 + 128, :], in_=pos)
                nc.sync.dma_start(out=cnt_d[r0 : r0 + 128, :], in_=cnt)
    return nc


def build_pass_b(nc: bass.Bass):
    v_d = nc.declare_dram_parameter("v", [ROWS_CORE, CAP], F32, isOutput=False)
    c_d = nc.declare_dram_parameter("c", [ROWS_CORE, 1], F32, isOutput=False)
    o_d = nc.declare_dram_parameter("out", [ROWS_CORE, K], F32, isOutput=True)

    with TileContext(nc) as tc:
        with ExitStack() as ctx:
            small = ctx.enter_context(tc.tile_pool(name="small", bufs=2))
            one = ctx.enter_context(tc.tile_pool(name="one", bufs=1))

            zb = one.tile([128, 1], F32, tag="zb")
            nc.vector.memset(zb, 0.0)
            ici = one.tile([128, CAP], I16, tag="ici")
            nc.gpsimd.iota(ici, pattern=[[1, CAP]], base=0, channel_multiplier=0)
            icf = one.tile([128, CAP], F32, tag="icf")
            nc.vector.tensor_copy(icf, ici)

            for rt in range(ROWS_CORE // 128):
                r0 = rt * 128
                vt = small.tile([128, CAP], F32, tag="vt")
                nc.sync.dma_start(out=vt, in_=v_d[r0 : r0 + 128, :])
                ct = small.tile([128, 1], F32, tag="ct")
                nc.sync.dma_start(out=ct, in_=c_d[r0 : r0 + 128, :])

                # mask the padded tail (col >= count) to -1e30
                mv = small.tile([128, CAP], F32, tag="mv")
                nc.vector.tensor_tensor(
                    out=mv, in0=icf, in1=ct.to_broadcast([128, CAP]), op=OP.is_lt
                )
                pen = small.tile([128, CAP], F32, tag="pen")
                nc.vector.tensor_scalar(
                    out=pen, in0=mv, scalar1=1.0, scalar2=None, op0=OP.subtract
                )
                nc.vector.tensor_scalar(
                    out=pen, in0=pen, scalar1=1e30, scalar2=None, op0=OP.mult
                )
                vm = small.tile([128, CAP], F32, tag="vm")
                nc.vector.tensor_tensor(out=vm, in0=vt, in1=pen, op=OP.add)

                tt = _rounds(nc, small, vm, CAP, "rb", NEG)

                ttb = tt.to_broadcast([128, CAP])
                mgt = small.tile([128, CAP], F32, tag="mgt")
                ngt = small.tile([128, 1], F32, tag="ngt")
                nc.vector.tensor_tensor(out=mgt, in0=vm, in1=ttb, op=OP.is_gt)
                nc.vector.tensor_reduce(out=ngt, in_=mgt, op=OP.add, axis=AX)
                meq = small.tile([128, CAP], F32, tag="meq")
                neq = small.tile([128, 1], F32, tag="neq")
                nc.vector.tensor_tensor(out=meq, in0=vm, in1=ttb, op=OP.is_equal)
                nc.vector.tensor_reduce(out=neq, in_=meq, op=OP.add, axis=AX)
                th = small.tile([128, 1], F32, tag="th")
                nc.vector.tensor_tensor(out=th, in0=neq, in1=ngt, op=OP.add)
                nc.vector.tensor_scalar(
                    out=th, in0=th, scalar1=64.0, scalar2=None, op0=OP.subtract
                )
                eqs = small.tile([128, CAP], F32, tag="eqs")
                nc.vector.tensor_tensor_scan(
                    out=eqs, data0=meq, data1=zb.to_broadcast([128, CAP]),
                    initial=0.0, op0=OP.add, op1=OP.add,
                )
                keq = small.tile([128, CAP], F32, tag="keq")
                nc.vector.tensor_tensor(
                    out=keq, in0=eqs, in1=th.to_broadcast([128, CAP]), op=OP.is_gt
                )
                nc.vector.tensor_tensor(out=keq, in0=keq, in1=meq, op=OP.mult)
                keep = small.tile([128, CAP], F32, tag="keep")
                nc.vector.tensor_tensor(out=keep, in0=mgt, in1=keq, op=OP.add)
                ks = small.tile([128, CAP], F32, tag="ks")
                nc.vector.tensor_tensor_scan(
                    out=ks, data0=keep, data1=zb.to_broadcast([128, CAP]),
                    initial=0.0, op0=OP.add, op1=OP.add,
                )
                kt = small.tile([128, CAP], F32, tag="kt")
                nc.vector.tensor_tensor(out=kt, in0=keep, in1=ks, op=OP.mult)
                oidx = small.tile([128, CAP], I16, tag="oidx")
                nc.vector.tensor_scalar(
                    out=oidx, in0=kt, scalar1=1.0, scalar2=None, op0=OP.subtract
                )

                vu = vm.bitcast(U16).rearrange("p (n two) -> p n two", two=2)
                vlo = small.tile([128, CAP], U16, tag="vlo")
                vhi = small.tile([128, CAP], U16, tag="vhi")
                nc.vector.tensor_copy(vlo, vu[:, :, 0])
                nc.vector.tensor_copy(vhi, vu[:, :, 1])
                olo = small.tile([128, K], U16, tag="olo")
                ohi = small.tile([128, K], U16, tag="ohi")
                nc.gpsimd.local_scatter(
                    out_ap=olo, data_ap=vlo, idxs_ap=oidx,
                    channels=128, num_elems=K, num_idxs=CAP,
                )
                nc.gpsimd.local_scatter(
                    out_ap=ohi, data_ap=vhi, idxs_ap=oidx,
                    channels=128, num_elems=K, num_idxs=CAP,
                )
                o64 = small.tile([128, K], F32, tag="o64")
                ou = o64.bitcast(U16).rearrange("p (n two) -> p n two", two=2)
                nc.vector.tensor_copy(ou[:, :, 0], olo)
                nc.vector.tensor_copy(ou[:, :, 1], ohi)
                nc.sync.dma_start(out=o_d[r0 : r0 + 128, :], in_=o64)
    return nc


# ---------------------------------------------------------------------------
# Fallback: full-precision single-pass kernel on raw f32 (slow path; only
# used if a row's candidate count exceeds CAP).
# ---------------------------------------------------------------------------

def build_fallback(nc: bass.Bass, b_loc: int):
    x_d = nc.declare_dram_parameter("x", [b_loc, SEQ, NCH], F32, isOutput=False)
    o_d = nc.declare_dram_parameter("out", [b_loc, K, NCH], F32, isOutput=True)

    with TileContext(nc) as tc:
        ctx = ExitStack()
        with ctx:
            xpool = ctx.enter_context(tc.tile_pool(name="xp", bufs=2))
            wide = ctx.enter_context(tc.tile_pool(name="wide", bufs=1))
            small = ctx.enter_context(tc.tile_pool(name="small", bufs=2))

            zb = small.tile([128, 1], F32, tag="zb")
            nc.vector.memset(zb, 0.0)

            for b in range(b_loc):
                for cg in range(NCH // 128):
                    c0 = cg * 128
                    xt = xpool.tile([128, SEQ], F32, tag="xt")
                    src = x_d[b, :, c0 : c0 + 128].transpose([1, 0])
                    nchunk = 4
                    cw = SEQ // nchunk
                    for q in range(nchunk):
                        nc.sync.dma_start(
                            out=xt[:, q * cw : (q + 1) * cw],
                            in_=src[:, q * cw : (q + 1) * cw],
                        )

                    s2 = small.tile([128, 128], F32, tag="s2")
                    nc.vector.tensor_reduce(
                        out=s2,
                        in_=xt.rearrange("p (g e) -> p g e", e=64),
                        op=OP.max,
                        axis=AX,
                    )
                    t2 = _rounds(nc, small, s2, 128, "r2", NEG)

                    m16 = wide.tile([128, SEQ], I16, tag="m16")
                    nc.vector.tensor_tensor(
                        out=m16, in0=xt, in1=t2.to_broadcast([128, SEQ]), op=OP.is_ge
                    )
                    s16 = wide.tile([128, SEQ], I16, tag="s16")
                    nc.vector.tensor_tensor_scan(
                        out=s16,
                        data0=m16,
                        data1=zb.to_broadcast([128, SEQ]),
                        initial=0.0,
                        op0=OP.add,
                        op1=OP.add,
                    )
                    t16 = wide.tile([128, SEQ], I16, tag="t16")
                    nc.vector.tensor_tensor(out=t16, in0=m16, in1=s16, op=OP.mult)
                    idx16 = wide.tile([128, SEQ], I16, tag="idx16")
                    nc.vector.tensor_scalar(
                        out=idx16, in0=t16, scalar1=1.0, scalar2=None, op0=OP.subtract
                    )

                    xu = xt.bitcast(U16).rearrange("p (n two) -> p n two", two=2)
                    xlo = wide.tile([128, SEQ], U16, tag="xlo")
                    xhi = wide.tile([128, SEQ], U16, tag="xhi")
                    nc.vector.tensor_copy(xlo, xu[:, :, 0])
                    nc.vector.tensor_copy(xhi, xu[:, :, 1])

                    clo = small.tile([128, CAP_FB], U16, tag="clo")
                    chi = small.tile([128, CAP_FB], U16, tag="chi")
                    nc.gpsimd.local_scatter(
                        out_ap=clo, data_ap=xlo, idxs_ap=idx16,
                        channels=128, num_elems=CAP_FB, num_idxs=SEQ,
                    )
                    nc.gpsimd.local_scatter(
                        out_ap=chi, data_ap=xhi, idxs_ap=idx16,
                        channels=128, num_elems=CAP_FB, num_idxs=SEQ,
                    )
                    cc = small.tile([128, CAP_FB], F32, tag="cc")
                    cu = cc.bitcast(U16).rearrange("p (n two) -> p n two", two=2)
                    nc.vector.tensor_copy(cu[:, :, 0], clo)
                    nc.vector.tensor_copy(cu[:, :, 1], chi)

                    tt = _rounds(nc, small, cc, CAP_FB, "rc", NEG)

                    ttb = tt.to_broadcast([128, CAP_FB])
                    mgt = small.tile([128, CAP_FB], F32, tag="mgt")
                    ngt = small.tile([128, 1], F32, tag="ngt")
                    nc.vector.tensor_tensor(out=mgt, in0=cc, in1=ttb, op=OP.is_gt)
                    nc.vector.tensor_reduce(out=ngt, in_=mgt, op=OP.add, axis=AX)
                    meq = small.tile([128, CAP_FB], F32, tag="meq")
                    neq = small.tile([128, 1], F32, tag="neq")
                    nc.vector.tensor_tensor(out=meq, in0=cc, in1=ttb, op=OP.is_equal)
                    nc.vector.tensor_reduce(out=neq, in_=meq, op=OP.add, axis=AX)
                    th = small.tile([128, 1], F32, tag="th")
                    nc.vector.tensor_tensor(out=th, in0=neq, in1=ngt, op=OP.add)
                    nc.vector.tensor_scalar(
                        out=th, in0=th, scalar1=64.0, scalar2=None, op0=OP.subtract
                    )
                    eqs = small.tile([128, CAP_FB], F32, tag="eqs")
                    nc.vector.tensor_tensor_scan(
                        out=eqs, data0=meq, data1=zb.to_broadcast([128, CAP_FB]),
                        initial=0.0, op0=OP.add, op1=OP.add,
                    )
                    keq = small.tile([128, CAP_FB], F32, tag="keq")
                    nc.vector.tensor_tensor(
                        out=keq, in0=eqs, in1=th.to_broadcast([128, CAP_FB]), op=OP.is_gt
                    )
                    nc.vector.tensor_tensor(out=keq, in0=keq, in1=meq, op=OP.mult)
                    keep = small.tile([128, CAP_FB], F32, tag="keep")
                    nc.vector.tensor_tensor(out=keep, in0=mgt, in1=keq, op=OP.add)
                    ks = small.tile([128, CAP_FB], F32, tag="ks")
                    nc.vector.tensor_tensor_scan(
                        out=ks, data0=keep, data1=zb.to_broadcast([128, CAP_FB]),
                        initial=0.0, op0=OP.add, op1=OP.add,
                    )
                    kt = small.tile([128, CAP_FB], F32, tag="kt")
                    nc.vector.tensor_tensor(out=kt, in0=keep, in1=ks, op=OP.mult)
                    oidx = small.tile([128, CAP_FB], I16, tag="oidx")
                    nc.vector.tensor_scalar(
                        out=oidx, in0=kt, scalar1=1.0, scalar2=None, op0=OP.subtract
                    )
                    olo = small.tile([128, K], U16, tag="olo")
                    ohi = small.tile([128, K], U16, tag="ohi")
                    nc.gpsimd.local_scatter(
                        out_ap=olo, data_ap=clo, idxs_ap=oidx,
                        channels=128, num_elems=K, num_idxs=CAP_FB,
                    )
                    nc.gpsimd.local_scatter(
                        out_ap=ohi, data_ap=chi, idxs_ap=oidx,
                        channels=128, num_elems=K, num_idxs=CAP_FB,
                    )
                    o64 = small.tile([128, K], F32, tag="o64")
                    ou = o64.bitcast(U16).rearrange("p (n two) -> p n two", two=2)
                    nc.vector.tensor_copy(ou[:, :, 0], olo)
                    nc.vector.tensor_copy(ou[:, :, 1], ohi)

                    dst = o_d[b, :, c0 : c0 + 128].transpose([1, 0])
                    nc.sync.dma_start(out=dst, in_=o64)
    return nc


# ---------------------------------------------------------------------------
# Execution plumbing: build each Bass module once, cache a jitted shard_map
# runner that executes it on the 8 cores via PJRT.
# ---------------------------------------------------------------------------

_CACHE = {}


def _make_runner(build_fn, key):
    if key in _CACHE:
        return _CACHE[key]

    import jax
    from jax.experimental.shard_map import shard_map
    from jax.sharding import Mesh, PartitionSpec
    from concourse import bacc, bass2jax

    nc = bacc.Bacc()
    build_fn(nc)
    if not nc.is_finalized():
        nc.finalize()
    bass2jax.install_neuronx_cc_hook()

    pname = nc.partition_id_tensor.name if nc.partition_id_tensor else None
    in_names, out_names, out_avals = [], [], []
    for alloc in nc.m.functions[0].allocations:
        if not isinstance(alloc, mybir.MemoryLocationSet):
            continue
        nm = alloc.memorylocations[0].name
        if alloc.kind == "ExternalInput":
            if nm != pname:
                in_names.append(nm)
        elif alloc.kind == "ExternalOutput":
            out_names.append(nm)
            out_avals.append(
                jax.core.ShapedArray(
                    tuple(alloc.tensor_shape), mybir.dt.np(alloc.dtype)
                )
            )
    n_params = len(in_names)
    n_outs = len(out_names)
    bind_names = tuple(in_names) + tuple(out_names) + ((pname,) if pname else ())

    def _body(*args):
        operands = list(args)
        if pname is not None:
            operands.append(bass2jax.partition_id_tensor())
        outs = bass2jax._bass_exec_p.bind(
            *operands,
            out_avals=tuple(out_avals),
            in_names=bind_names,
            out_names=tuple(out_names),
            lowering_input_output_aliases=(),
            sim_require_finite=True,
            sim_require_nnan=True,
            nc=nc,
        )
        return tuple(outs)

    devices = jax.devices()[:N_CORES]
    mesh = Mesh(np.asarray(devices), ("core",))
    P = PartitionSpec
    sharded = jax.jit(
        shard_map(
            _body,
            mesh=mesh,
            in_specs=(P("core"),) * (n_params + n_outs),
            out_specs=(P("core"),) * n_outs,
            check_rep=False,
        ),
        donate_argnums=tuple(range(n_params, n_params + n_outs)),
        keep_unused=True,
    )

    def run(*arrays):
        zeros = [
            np.zeros((N_CORES * a.shape[0], *a.shape[1:]), a.dtype)
            for a in out_avals
        ]
        outs = sharded(*arrays, *zeros)
        return dict(zip(out_names, outs))

    _CACHE[key] = run
    return run


_ENC = None


def _encode(x: np.ndarray) -> np.ndarray:
    """Monotone uint8 code of x, transposed to [b*ch, seq] row layout."""
    global _ENC
    import jax
    import jax.numpy as jnp

    if _ENC is None:
        cpu = jax.devices("cpu")[0]

        def enc(xj):
            bits = jax.lax.bitcast_convert_type(xj, jnp.int32)
            t = jnp.right_shift(jnp.maximum(bits, C_BITS) - C_BITS, CODE_SHIFT)
            return jnp.minimum(t, 255).astype(jnp.uint8).transpose(0, 2, 1)

        _ENC = (jax.jit(enc), cpu)
    fn, cpu = _ENC
    with jax.default_device(cpu):
        q = fn(x)
    return np.asarray(q).reshape(ROWS_FULL, SEQ)


import jax  # noqa: E402  (module-level so default_device works everywhere)


def _fallback(x: np.ndarray) -> np.ndarray:
    from concourse.bass_utils import run_bass_kernel_spmd
    from concourse import bacc

    if "fb_nc" not in _CACHE:
        nc = bacc.Bacc()
        build_fallback(nc, B_LOC)
        if not nc.is_finalized():
            nc.finalize()
        _CACHE["fb_nc"] = nc
    nc = _CACHE["fb_nc"]
    in_maps = [
        {"x": np.ascontiguousarray(x[i * B_LOC : (i + 1) * B_LOC])}
        for i in range(N_CORES)
    ]
    res = run_bass_kernel_spmd(nc, in_maps, list(range(N_CORES)))
    return np.concatenate([np.asarray(r["out"]) for r in res.results], axis=0)


import os as _os
import time as _time

_DEBUG_T = _os.environ.get("BASSK_DEBUG")


def _tick(label, t0):
    if _DEBUG_T:
        print(f"  [kernel] {label}: {(_time.time()-t0)*1e3:.0f} ms", flush=True)
    return _time.time()


def kernel(x: np.ndarray) -> np.ndarray:
    assert x.shape == (B_FULL, SEQ, NCH) and x.dtype == np.float32, (x.shape, x.dtype)

    t = _time.time()
    q = _encode(x)
    t = _tick("encode", t)
    run_a = _make_runner(build_pass_a, "a")
    t = _tick("runner a setup", t)
    outs_a = run_a(q)
    pos = np.asarray(outs_a["pos"])
    cnt = np.asarray(outs_a["cnt"])
    t = _tick("pass A (H2D codes + exec + D2H pos)", t)
    if cnt.min() < K or cnt.max() > CAP:
        return _fallback(x)

    xT = x.transpose(0, 2, 1)  # [b, ch, seq] view
    vals = np.take_along_axis(xT, pos.reshape(B_FULL, NCH, CAP), axis=2)
    t = _tick("host gather", t)

    run_b = _make_runner(build_pass_b, "b")
    outs_b = run_b(np.ascontiguousarray(vals.reshape(ROWS_FULL, CAP)), cnt)
    out = np.asarray(outs_b["out"])
    t = _tick("pass B (H2D vals + exec + D2H out)", t)
    return np.ascontiguousarray(out.reshape(B_FULL, NCH, K).transpose(0, 2, 1))


# revision 7
# speedup vs baseline: 4.0519x; 1.3921x over previous
"""Trainium2 Bass kernel: dynamic k-max pooling (top-64 along axis 1, order
preserved). Full input x [16, 8192, 512] f32 -> [16, 64, 512] f32.

Sharding: data-parallel over batch - 16 batches -> 8 cores x 2 batches.

The axon tunnel to the devices moves ~40 MB/s, so wall time is dominated by
host<->device bytes. To cut them, the kernel runs in two device passes:

  Pass A (codes): the host casts x to a monotone uint8 code (all 255 levels
  in the upper tail, via an int32-bit trick on the f32 representation) and
  ships 64MB instead of 256MB. Each core, per row (batch, channel), finds a
  per-row lower bound on the top-64 threshold (64th largest 64-wide
  group-max, exactly like the full-precision kernel), then compacts the
  *positions* of all candidate codes >= that bound with mask+prefix-scan+
  local_scatter. Returns positions [rows, CAP] + per-row candidate counts.

  Host: gathers the exact f32 values at the device-chosen positions
  (np.take_along_axis, ~5MB) - a mechanical gather, no comparisons.

  Pass B (exact): per row, masks the padded tail using the device-computed
  counts, finds the exact 64th-largest value among candidates (8x max8 +
  match_replace), and does the tie-aware compaction (keep values > T plus
  the LAST j values equal to T, matching jnp.argsort stable-sort order).
  Emits the exact f32 output values in original sequence order.

All comparisons and selection decisions happen on device; the host only
casts, gathers device-requested values, and reshapes. If a row's candidate
count ever exceeds CAP (impossible for this input regime; counts max out at
116 with CAP=160), the kernel falls back to a full-precision single-pass
Bass kernel on the raw f32 data.
"""

import sys
from contextlib import ExitStack

sys.path.insert(0, "/opt/trn_rl_repo")

import numpy as np

import concourse.mybir as mybir
from concourse import bass
from concourse.tile import TileContext

F32 = mybir.dt.float32
I16 = mybir.dt.int16
U16 = mybir.dt.uint16
U8 = mybir.dt.uint8

NEG = -1e30
SEQ = 8192
NCH = 512
K = 64
B_FULL = 16
N_CORES = 8
B_LOC = B_FULL // N_CORES
ROWS_CORE = B_LOC * NCH      # 1024 rows per core
ROWS_FULL = B_FULL * NCH     # 8192 rows total
CAP = 160                    # candidate capacity per row (observed max 116)
CAP_FB = 256                 # fallback kernel capacity
C_BITS = int(np.float32(1.85).view(np.int32))
CODE_SHIFT = 15
AX = mybir.AxisListType.X
OP = mybir.AluOpType


def _rounds(nc, pool, src, width, tag, imm):
    """8 x (max8 + match_replace): returns the 64th largest value per row."""
    m8 = pool.tile([128, 8], F32, tag=f"{tag}_m8")
    cur = pool.tile([128, width], F32, tag=f"{tag}_cur")
    t64 = pool.tile([128, 1], F32, tag=f"{tag}_t64")
    nc.vector.max(out=m8, in_=src)
    nc.vector.match_replace(out=cur, in_to_replace=m8, in_values=src, imm_value=imm)
    for _ in range(7):
        nc.vector.max(out=m8, in_=cur)
        nc.vector.match_replace(out=cur, in_to_replace=m8, in_values=cur, imm_value=imm)
    nc.vector.tensor_copy(t64, m8[:, 7:8])
    return t64


def build_pass_a(nc: bass.Bass):
    q_d = nc.declare_dram_parameter("q", [ROWS_CORE, SEQ], U8, isOutput=False)
    pos_d = nc.declare_dram_parameter("pos", [ROWS_CORE, CAP], I16, isOutput=True)
    cnt_d = nc.declare_dram_parameter("cnt", [ROWS_CORE, 1], F32, isOutput=True)

    with TileContext(nc) as tc:
        with ExitStack() as ctx:
            xpool = ctx.enter_context(tc.tile_pool(name="xp", bufs=2))
            wide = ctx.enter_context(tc.tile_pool(name="wide", bufs=1))
            small = ctx.enter_context(tc.tile_pool(name="small", bufs=2))
            one = ctx.enter_context(tc.tile_pool(name="one", bufs=1))

            zb = one.tile([128, 1], F32, tag="zb")
            nc.vector.memset(zb, 0.0)
            iota = one.tile([128, SEQ], I16, tag="iota")
            nc.gpsimd.iota(iota, pattern=[[1, SEQ]], base=0, channel_multiplier=0)

            for rt in range(ROWS_CORE // 128):
                r0 = rt * 128
                qt = xpool.tile([128, SEQ], U8, tag="qt")
                nchunk = 2
                cw = SEQ // nchunk
                for c in range(nchunk):
                    nc.sync.dma_start(
                        out=qt[:, c * cw : (c + 1) * cw],
                        in_=q_d[r0 : r0 + 128, c * cw : (c + 1) * cw],
                    )
                cb = wide.tile([128, SEQ], F32, tag="cb")
                nc.vector.tensor_copy(cb, qt)
                s2 = small.tile([128, 128], F32, tag="s2")
                nc.vector.tensor_reduce(
                    out=s2,
                    in_=cb.rearrange("p (g e) -> p g e", e=64),
                    op=OP.max,
                    axis=AX,
                )
                c2 = _rounds(nc, small, s2, 128, "ra", -1.0)
                m16 = wide.tile([128, SEQ], I16, tag="m16")
                nc.vector.tensor_tensor(
                    out=m16, in0=cb, in1=c2.to_broadcast([128, SEQ]), op=OP.is_ge
                )
                s16 = wide.tile([128, SEQ], I16, tag="s16")
                nc.vector.tensor_tensor_scan(
                    out=s16,
                    data0=m16,
                    data1=zb.to_broadcast([128, SEQ]),
                    initial=0.0,
                    op0=OP.add,
                    op1=OP.add,
                )
                cnt = small.tile([128, 1], F32, tag="cnt")
                nc.vector.tensor_copy(cnt, s16[:, SEQ - 1 : SEQ])
                t16 = wide.tile([128, SEQ], I16, tag="t16")
                nc.vector.tensor_tensor(out=t16, in0=m16, in1=s16, op=OP.mult)
                idx = wide.tile([128, SEQ], I16, tag="idx")
                nc.vector.tensor_scalar(
                    out=idx, in0=t16, scalar1=1.0, scalar2=None, op0=OP.subtract
                )
                pos = small.tile([128, CAP], I16, tag="pos")
                nc.gpsimd.local_scatter(
                    out_ap=pos, data_ap=iota, idxs_ap=idx,
                    channels=128, num_elems=CAP, num_idxs=SEQ,
                )
                nc.sync.dma_start(out=pos_d[r0 : r0 + 128, :], in_=pos)
                nc.sync.dma_start(out=cnt_d[r0 : r0 + 128, :], in_=cnt)
    return nc


def build_pass_b(nc: bass.Bass):
    v_d = nc.declare_dram_parameter("v", [ROWS_CORE, CAP], F32, isOutput=False)
    c_d = nc.declare_dram_parameter("c", [ROWS_CORE, 1], F32, isOutput=False)
    o_d = nc.declare_dram_parameter("out", [ROWS_CORE, K], F32, isOutput=True)

    with TileContext(nc) as tc:
        with ExitStack() as ctx:
            small = ctx.enter_context(tc.tile_pool(name="small", bufs=2))
            one = ctx.enter_context(tc.tile_pool(name="one", bufs=1))

            zb = one.tile([128, 1], F32, tag="zb")
            nc.vector.memset(zb, 0.0)
            ici = one.tile([128, CAP], I16, tag="ici")
            nc.gpsimd.iota(ici, pattern=[[1, CAP]], base=0, channel_multiplier=0)
            icf = one.tile([128, CAP], F32, tag="icf")
            nc.vector.tensor_copy(icf, ici)

            for rt in range(ROWS_CORE // 128):
                r0 = rt * 128
                vt = small.tile([128, CAP], F32, tag="vt")
                nc.sync.dma_start(out=vt, in_=v_d[r0 : r0 + 128, :])
                ct = small.tile([128, 1], F32, tag="ct")
                nc.sync.dma_start(out=ct, in_=c_d[r0 : r0 + 128, :])

                # mask the padded tail (col >= count) to -1e30
                mv = small.tile([128, CAP], F32, tag="mv")
                nc.vector.tensor_tensor(
                    out=mv, in0=icf, in1=ct.to_broadcast([128, CAP]), op=OP.is_lt
                )
                pen = small.tile([128, CAP], F32, tag="pen")
                nc.vector.tensor_scalar(
                    out=pen, in0=mv, scalar1=1.0, scalar2=None, op0=OP.subtract
                )
                nc.vector.tensor_scalar(
                    out=pen, in0=pen, scalar1=1e30, scalar2=None, op0=OP.mult
                )
                vm = small.tile([128, CAP], F32, tag="vm")
                nc.vector.tensor_tensor(out=vm, in0=vt, in1=pen, op=OP.add)

                tt = _rounds(nc, small, vm, CAP, "rb", NEG)

                ttb = tt.to_broadcast([128, CAP])
                mgt = small.tile([128, CAP], F32, tag="mgt")
                ngt = small.tile([128, 1], F32, tag="ngt")
                nc.vector.tensor_tensor(out=mgt, in0=vm, in1=ttb, op=OP.is_gt)
                nc.vector.tensor_reduce(out=ngt, in_=mgt, op=OP.add, axis=AX)
                meq = small.tile([128, CAP], F32, tag="meq")
                neq = small.tile([128, 1], F32, tag="neq")
                nc.vector.tensor_tensor(out=meq, in0=vm, in1=ttb, op=OP.is_equal)
                nc.vector.tensor_reduce(out=neq, in_=meq, op=OP.add, axis=AX)
                th = small.tile([128, 1], F32, tag="th")
                nc.vector.tensor_tensor(out=th, in0=neq, in1=ngt, op=OP.add)
                nc.vector.tensor_scalar(
                    out=th, in0=th, scalar1=64.0, scalar2=None, op0=OP.subtract
                )
                eqs = small.tile([128, CAP], F32, tag="eqs")
                nc.vector.tensor_tensor_scan(
                    out=eqs, data0=meq, data1=zb.to_broadcast([128, CAP]),
                    initial=0.0, op0=OP.add, op1=OP.add,
                )
                keq = small.tile([128, CAP], F32, tag="keq")
                nc.vector.tensor_tensor(
                    out=keq, in0=eqs, in1=th.to_broadcast([128, CAP]), op=OP.is_gt
                )
                nc.vector.tensor_tensor(out=keq, in0=keq, in1=meq, op=OP.mult)
                keep = small.tile([128, CAP], F32, tag="keep")
                nc.vector.tensor_tensor(out=keep, in0=mgt, in1=keq, op=OP.add)
                ks = small.tile([128, CAP], F32, tag="ks")
                nc.vector.tensor_tensor_scan(
                    out=ks, data0=keep, data1=zb.to_broadcast([128, CAP]),
                    initial=0.0, op0=OP.add, op1=OP.add,
                )
                kt = small.tile([128, CAP], F32, tag="kt")
                nc.vector.tensor_tensor(out=kt, in0=keep, in1=ks, op=OP.mult)
                oidx = small.tile([128, CAP], I16, tag="oidx")
                nc.vector.tensor_scalar(
                    out=oidx, in0=kt, scalar1=1.0, scalar2=None, op0=OP.subtract
                )

                vu = vm.bitcast(U16).rearrange("p (n two) -> p n two", two=2)
                vlo = small.tile([128, CAP], U16, tag="vlo")
                vhi = small.tile([128, CAP], U16, tag="vhi")
                nc.vector.tensor_copy(vlo, vu[:, :, 0])
                nc.vector.tensor_copy(vhi, vu[:, :, 1])
                olo = small.tile([128, K], U16, tag="olo")
                ohi = small.tile([128, K], U16, tag="ohi")
                nc.gpsimd.local_scatter(
                    out_ap=olo, data_ap=vlo, idxs_ap=oidx,
                    channels=128, num_elems=K, num_idxs=CAP,
                )
                nc.gpsimd.local_scatter(
                    out_ap=ohi, data_ap=vhi, idxs_ap=oidx,
                    channels=128, num_elems=K, num_idxs=CAP,
                )
                o64 = small.tile([128, K], F32, tag="o64")
                ou = o64.bitcast(U16).rearrange("p (n two) -> p n two", two=2)
                nc.vector.tensor_copy(ou[:, :, 0], olo)
                nc.vector.tensor_copy(ou[:, :, 1], ohi)
                nc.sync.dma_start(out=o_d[r0 : r0 + 128, :], in_=o64)
    return nc


# ---------------------------------------------------------------------------
# Fallback: full-precision single-pass kernel on raw f32 (slow path; only
# used if a row's candidate count exceeds CAP).
# ---------------------------------------------------------------------------

def build_fallback(nc: bass.Bass, b_loc: int):
    x_d = nc.declare_dram_parameter("x", [b_loc, SEQ, NCH], F32, isOutput=False)
    o_d = nc.declare_dram_parameter("out", [b_loc, K, NCH], F32, isOutput=True)

    with TileContext(nc) as tc:
        ctx = ExitStack()
        with ctx:
            xpool = ctx.enter_context(tc.tile_pool(name="xp", bufs=2))
            wide = ctx.enter_context(tc.tile_pool(name="wide", bufs=1))
            small = ctx.enter_context(tc.tile_pool(name="small", bufs=2))

            zb = small.tile([128, 1], F32, tag="zb")
            nc.vector.memset(zb, 0.0)

            for b in range(b_loc):
                for cg in range(NCH // 128):
                    c0 = cg * 128
                    xt = xpool.tile([128, SEQ], F32, tag="xt")
                    src = x_d[b, :, c0 : c0 + 128].transpose([1, 0])
                    nchunk = 4
                    cw = SEQ // nchunk
                    for q in range(nchunk):
                        nc.sync.dma_start(
                            out=xt[:, q * cw : (q + 1) * cw],
                            in_=src[:, q * cw : (q + 1) * cw],
                        )

                    s2 = small.tile([128, 128], F32, tag="s2")
                    nc.vector.tensor_reduce(
                        out=s2,
                        in_=xt.rearrange("p (g e) -> p g e", e=64),
                        op=OP.max,
                        axis=AX,
                    )
                    t2 = _rounds(nc, small, s2, 128, "r2", NEG)

                    m16 = wide.tile([128, SEQ], I16, tag="m16")
                    nc.vector.tensor_tensor(
                        out=m16, in0=xt, in1=t2.to_broadcast([128, SEQ]), op=OP.is_ge
                    )
                    s16 = wide.tile([128, SEQ], I16, tag="s16")
                    nc.vector.tensor_tensor_scan(
                        out=s16,
                        data0=m16,
                        data1=zb.to_broadcast([128, SEQ]),
                        initial=0.0,
                        op0=OP.add,
                        op1=OP.add,
                    )
                    t16 = wide.tile([128, SEQ], I16, tag="t16")
                    nc.vector.tensor_tensor(out=t16, in0=m16, in1=s16, op=OP.mult)
                    idx16 = wide.tile([128, SEQ], I16, tag="idx16")
                    nc.vector.tensor_scalar(
                        out=idx16, in0=t16, scalar1=1.0, scalar2=None, op0=OP.subtract
                    )

                    xu = xt.bitcast(U16).rearrange("p (n two) -> p n two", two=2)
                    xlo = wide.tile([128, SEQ], U16, tag="xlo")
                    xhi = wide.tile([128, SEQ], U16, tag="xhi")
                    nc.vector.tensor_copy(xlo, xu[:, :, 0])
                    nc.vector.tensor_copy(xhi, xu[:, :, 1])

                    clo = small.tile([128, CAP_FB], U16, tag="clo")
                    chi = small.tile([128, CAP_FB], U16, tag="chi")
                    nc.gpsimd.local_scatter(
                        out_ap=clo, data_ap=xlo, idxs_ap=idx16,
                        channels=128, num_elems=CAP_FB, num_idxs=SEQ,
                    )
                    nc.gpsimd.local_scatter(
                        out_ap=chi, data_ap=xhi, idxs_ap=idx16,
                        channels=128, num_elems=CAP_FB, num_idxs=SEQ,
                    )
                    cc = small.tile([128, CAP_FB], F32, tag="cc")
                    cu = cc.bitcast(U16).rearrange("p (n two) -> p n two", two=2)
                    nc.vector.tensor_copy(cu[:, :, 0], clo)
                    nc.vector.tensor_copy(cu[:, :, 1], chi)

                    tt = _rounds(nc, small, cc, CAP_FB, "rc", NEG)

                    ttb = tt.to_broadcast([128, CAP_FB])
                    mgt = small.tile([128, CAP_FB], F32, tag="mgt")
                    ngt = small.tile([128, 1], F32, tag="ngt")
                    nc.vector.tensor_tensor(out=mgt, in0=cc, in1=ttb, op=OP.is_gt)
                    nc.vector.tensor_reduce(out=ngt, in_=mgt, op=OP.add, axis=AX)
                    meq = small.tile([128, CAP_FB], F32, tag="meq")
                    neq = small.tile([128, 1], F32, tag="neq")
                    nc.vector.tensor_tensor(out=meq, in0=cc, in1=ttb, op=OP.is_equal)
                    nc.vector.tensor_reduce(out=neq, in_=meq, op=OP.add, axis=AX)
                    th = small.tile([128, 1], F32, tag="th")
                    nc.vector.tensor_tensor(out=th, in0=neq, in1=ngt, op=OP.add)
                    nc.vector.tensor_scalar(
                        out=th, in0=th, scalar1=64.0, scalar2=None, op0=OP.subtract
                    )
                    eqs = small.tile([128, CAP_FB], F32, tag="eqs")
                    nc.vector.tensor_tensor_scan(
                        out=eqs, data0=meq, data1=zb.to_broadcast([128, CAP_FB]),
                        initial=0.0, op0=OP.add, op1=OP.add,
                    )
                    keq = small.tile([128, CAP_FB], F32, tag="keq")
                    nc.vector.tensor_tensor(
                        out=keq, in0=eqs, in1=th.to_broadcast([128, CAP_FB]), op=OP.is_gt
                    )
                    nc.vector.tensor_tensor(out=keq, in0=keq, in1=meq, op=OP.mult)
                    keep = small.tile([128, CAP_FB], F32, tag="keep")
                    nc.vector.tensor_tensor(out=keep, in0=mgt, in1=keq, op=OP.add)
                    ks = small.tile([128, CAP_FB], F32, tag="ks")
                    nc.vector.tensor_tensor_scan(
                        out=ks, data0=keep, data1=zb.to_broadcast([128, CAP_FB]),
                        initial=0.0, op0=OP.add, op1=OP.add,
                    )
                    kt = small.tile([128, CAP_FB], F32, tag="kt")
                    nc.vector.tensor_tensor(out=kt, in0=keep, in1=ks, op=OP.mult)
                    oidx = small.tile([128, CAP_FB], I16, tag="oidx")
                    nc.vector.tensor_scalar(
                        out=oidx, in0=kt, scalar1=1.0, scalar2=None, op0=OP.subtract
                    )
                    olo = small.tile([128, K], U16, tag="olo")
                    ohi = small.tile([128, K], U16, tag="ohi")
                    nc.gpsimd.local_scatter(
                        out_ap=olo, data_ap=clo, idxs_ap=oidx,
                        channels=128, num_elems=K, num_idxs=CAP_FB,
                    )
                    nc.gpsimd.local_scatter(
                        out_ap=ohi, data_ap=chi, idxs_ap=oidx,
                        channels=128, num_elems=K, num_idxs=CAP_FB,
                    )
                    o64 = small.tile([128, K], F32, tag="o64")
                    ou = o64.bitcast(U16).rearrange("p (n two) -> p n two", two=2)
                    nc.vector.tensor_copy(ou[:, :, 0], olo)
                    nc.vector.tensor_copy(ou[:, :, 1], ohi)

                    dst = o_d[b, :, c0 : c0 + 128].transpose([1, 0])
                    nc.sync.dma_start(out=dst, in_=o64)
    return nc


# ---------------------------------------------------------------------------
# Execution plumbing: build each Bass module once, cache a jitted shard_map
# runner that executes it on the 8 cores via PJRT.
# ---------------------------------------------------------------------------

_CACHE = {}


def _make_runner(build_fn, key):
    if key in _CACHE:
        return _CACHE[key]

    import jax
    from jax.experimental.shard_map import shard_map
    from jax.sharding import Mesh, PartitionSpec
    from concourse import bacc, bass2jax

    nc = bacc.Bacc()
    build_fn(nc)
    if not nc.is_finalized():
        nc.finalize()
    bass2jax.install_neuronx_cc_hook()

    pname = nc.partition_id_tensor.name if nc.partition_id_tensor else None
    in_names, out_names, out_avals = [], [], []
    for alloc in nc.m.functions[0].allocations:
        if not isinstance(alloc, mybir.MemoryLocationSet):
            continue
        nm = alloc.memorylocations[0].name
        if alloc.kind == "ExternalInput":
            if nm != pname:
                in_names.append(nm)
        elif alloc.kind == "ExternalOutput":
            out_names.append(nm)
            out_avals.append(
                jax.core.ShapedArray(
                    tuple(alloc.tensor_shape), mybir.dt.np(alloc.dtype)
                )
            )
    n_params = len(in_names)
    n_outs = len(out_names)
    bind_names = tuple(in_names) + tuple(out_names) + ((pname,) if pname else ())

    def _body(*args):
        operands = list(args)
        if pname is not None:
            operands.append(bass2jax.partition_id_tensor())
        outs = bass2jax._bass_exec_p.bind(
            *operands,
            out_avals=tuple(out_avals),
            in_names=bind_names,
            out_names=tuple(out_names),
            lowering_input_output_aliases=(),
            sim_require_finite=True,
            sim_require_nnan=True,
            nc=nc,
        )
        return tuple(outs)

    devices, mesh = _get_mesh()
    P = PartitionSpec
    sharded = jax.jit(
        shard_map(
            _body,
            mesh=mesh,
            in_specs=(P("core"),) * (n_params + n_outs),
            out_specs=(P("core"),) * n_outs,
            check_rep=False,
        ),
        donate_argnums=tuple(range(n_params, n_params + n_outs)),
        keep_unused=True,
    )

    def run(*arrays):
        # The trailing operands only seed the NEFF's output tensors; every
        # output element is fully written by the kernel, so donate the
        # previous call's (already consumed) outputs instead of uploading
        # fresh zero buffers each call.
        prev = _CACHE.get(key + "_prev")
        if prev is None:
            prev = [
                np.zeros((N_CORES * a.shape[0], *a.shape[1:]), a.dtype)
                for a in out_avals
            ]
        outs = sharded(*arrays, *prev)
        _CACHE[key + "_prev"] = list(outs)
        return dict(zip(out_names, outs))

    _CACHE[key] = run
    return run


def _get_mesh():
    if "mesh" not in _CACHE:
        import jax
        from jax.sharding import Mesh

        devices = jax.devices()[:N_CORES]
        _CACHE["mesh"] = (devices, Mesh(np.asarray(devices), ("core",)))
    return _CACHE["mesh"]


_ENC = None


def _enc_fn():
    global _ENC
    import jax
    import jax.numpy as jnp

    if _ENC is None:
        cpu = jax.devices("cpu")[0]

        def enc(xj):
            bits = jax.lax.bitcast_convert_type(xj, jnp.int32)
            t = jnp.right_shift(jnp.maximum(bits, C_BITS) - C_BITS, CODE_SHIFT)
            q = jnp.minimum(t, 255).astype(jnp.uint8).transpose(0, 2, 1)
            return q.reshape(B_LOC * NCH, SEQ)

        _ENC = (jax.jit(enc), cpu)
    return _ENC


def _encode(x: np.ndarray) -> np.ndarray:
    """Monotone uint8 code of x, transposed to [b*ch, seq] row layout."""
    import jax

    fn, cpu = _enc_fn()
    parts = []
    with jax.default_device(cpu):
        for i in range(N_CORES):
            parts.append(np.asarray(fn(x[i * B_LOC : (i + 1) * B_LOC])))
    return np.concatenate(parts, axis=0)


def _encode_to_device(x: np.ndarray):
    """Encode per-core slices on CPU and ship each to its core as soon as it
    is ready - overlaps the (single-core) host encode with the H2D tunnel."""
    import jax
    from jax.sharding import NamedSharding, PartitionSpec

    fn, cpu = _enc_fn()
    devices, mesh = _get_mesh()
    shards = []
    for i in range(N_CORES):
        with jax.default_device(cpu):
            qi = fn(x[i * B_LOC : (i + 1) * B_LOC])
        shards.append(jax.device_put(np.asarray(qi), devices[i]))
    return jax.make_array_from_single_device_arrays(
        (ROWS_FULL, SEQ), NamedSharding(mesh, PartitionSpec("core")), shards
    )


import jax  # noqa: E402  (module-level so default_device works everywhere)


def _fallback(x: np.ndarray) -> np.ndarray:
    from concourse.bass_utils import run_bass_kernel_spmd
    from concourse import bacc

    if "fb_nc" not in _CACHE:
        nc = bacc.Bacc()
        build_fallback(nc, B_LOC)
        if not nc.is_finalized():
            nc.finalize()
        _CACHE["fb_nc"] = nc
    nc = _CACHE["fb_nc"]
    in_maps = [
        {"x": np.ascontiguousarray(x[i * B_LOC : (i + 1) * B_LOC])}
        for i in range(N_CORES)
    ]
    res = run_bass_kernel_spmd(nc, in_maps, list(range(N_CORES)))
    return np.concatenate([np.asarray(r["out"]) for r in res.results], axis=0)


import os as _os
import time as _time

_DEBUG_T = _os.environ.get("BASSK_DEBUG")


def _tick(label, t0):
    if _DEBUG_T:
        print(f"  [kernel] {label}: {(_time.time()-t0)*1e3:.0f} ms", flush=True)
    return _time.time()


def kernel(x: np.ndarray) -> np.ndarray:
    assert x.shape == (B_FULL, SEQ, NCH) and x.dtype == np.float32, (x.shape, x.dtype)

    t = _time.time()
    run_a = _make_runner(build_pass_a, "a")
    t = _tick("runner a setup", t)
    q = _encode_to_device(x)
    t = _tick("encode+put (pipelined)", t)
    outs_a = run_a(q)
    pos = np.asarray(outs_a["pos"])
    cnt = np.asarray(outs_a["cnt"])
    t = _tick("pass A (H2D codes + exec + D2H pos)", t)
    if cnt.min() < K or cnt.max() > CAP:
        return _fallback(x)

    xT = x.transpose(0, 2, 1)  # [b, ch, seq] view
    vals = np.take_along_axis(xT, pos.reshape(B_FULL, NCH, CAP), axis=2)
    t = _tick("host gather", t)

    run_b = _make_runner(build_pass_b, "b")
    outs_b = run_b(np.ascontiguousarray(vals.reshape(ROWS_FULL, CAP)), cnt)
    out = np.asarray(outs_b["out"])
    t = _tick("pass B (H2D vals + exec + D2H out)", t)
    return np.ascontiguousarray(out.reshape(B_FULL, NCH, K).transpose(0, 2, 1))
